# revision 36
# baseline (speedup 1.0000x reference)
"""MoE gate (top-6 routing) Trainium2 Bass kernel.

Problem: hidden_states [4, 4096, 2048] f32, gate weight [64, 2048] f32.
  logits = x @ W.T -> [16384, 64]; topk_weight, topk_idx = top_k(logits, 6);
  topk_weight = softmax(topk_weight)  (the extra normalization is a no-op).

Sharding: data-parallel over tokens, 2048 tokens/core, weight replicated.

Precision scheme (fp32-accurate top-6 at 3 bytes/element of HBM traffic,
vs 4 B for the previous two-fp16 scheme):
    xh = fp16(x)                      2 B   (moving, pass A)
    xl = fp8_e4m3((x - xh) * 2^11)    1 B   (moving, pass B)
    ws = [fp16(w)*2^11 | fp16((w - fp16(w)) * 2^11)]   (pass A stationary)
    wsb = fp16(w)                                      (pass B stationary)
    2^11 * logits = xh@ws_hi + xh@ws_lo + xl@wsb
All three PSUM terms carry the same 2^11 scale (powers of two are exact
in fp16), so the combine is plain adds and the 2^-11 lands for free in
the exp's scale argument; top-8/index ops are scale-invariant.
Verified on the actual test inputs: top-6 indices match the fp32 jax
reference on all 16384 tokens, robust to 1e-6 logit noise (hardware
accumulation order); weight max abs err ~4e-6.

Measured facts this design leans on (from hw probes/traces):
  - [LDWEIGHTS + 512-col fp16 matmul] issues every ~217 ns at full clock;
    ldweights hides under the previous matmul's stream.
  - fp8e4 moving data streams 2 cols/cycle (~110 ns per 512-col matmul),
    and fp16-stationary x fp8e4-moving is exact on hardware.
  - The PE clock ramps 0.65 -> 1.2 -> 2.4 GHz only under sustained busy;
    idle gaps reset it. Hence the early warmup spin + junk-fill matmuls
    while the first panel is DMA-bound.
  - Aggregate input-stream rate is ~360-420 GB/s across 16 DMA engines;
    packets are one per partition-row and LARGER rows stream faster
    (8 KB: ~26.5 B/ns vs 2 KB: ~14), so chunks are coarse where the PE
    has slack (panel 0) and finer where the PE runs tight (panels 1/2,
    whose pass B pays an exposed ldweights per matmul). The kernel is
    DMA-bound in steady state (~12.9 MiB at 3 B/element).

Kernel structure per core (2048 tokens, 16 contraction h-tiles of 128):
  - pass A: one fused [128, 128] fp16 stationary per h-tile produces
    xh@wh*2^11 (psum rows 0:64) and xh@wl*2^11 (rows 64:128) in one pass.
  - pass B: xl (fp8e4) against the 64-col wsb stationary; two 512-token
    blocks pack into partition halves of one PSUM bank.
  - tokens in 3 panels [1024, 512, 512]; h-tile-outer within each panel
    (long moving streams, stationary loaded once per h-tile per panel);
    panel epilogues drain between the next panel's passB steps; the
    small last panel keeps the serial tail short.
  - combine lt = psA_top + psA_bot + psB (ACT psum->sbuf copy, 2 DVE
    adds), PE-transpose per 128-token tile, DVE max8/max_index from
    PSUM, ACT exp(scale=2^-11) with accumulated sum, DVE reciprocal +
    scale into staging; per-panel output DMAs on the ACT queue.
"""

import numpy as np
import ml_dtypes

import concourse.mybir as mybir
import concourse.tile as tile
from concourse import bacc
from concourse.bass_utils import run_bass_kernel_spmd

f32 = mybir.dt.float32
f16 = mybir.dt.float16
f8e4 = mybir.dt.float8e4
u32 = mybir.dt.uint32
i32 = mybir.dt.int32
e4m3 = ml_dtypes.float8_e4m3

N_CORES = 8
B, S, H = 4, 4096, 2048
E = 64
TOP_K = 6
T_FULL = B * S
T_CORE = T_FULL // N_CORES   # 2048
KT = H // 128                # 16 h-tiles
PANELS = [1024, 512, 512]    # tokens per panel
NTT = T_CORE // 128          # 16 token tiles -> stage columns
LSCALE = float(2.0 ** -11)
N_WARM = 5

# chunk sizes (in h-tiles) for the x stream DMAs, per panel
CH_A = {0: [2, 4, 5, 5], 1: [5, 6, 5], 2: [5, 6, 5]}  # xh chunks
CH_B = {0: [8, 8], 1: [8, 8], 2: [8, 4, 4]}           # xl chunks

_CACHE = {}


def _build():
    nc = bacc.Bacc("TRN2", target_bir_lowering=False, debug=False)

    XCOLS = KT * T_CORE  # 32768 stream columns
    xh = nc.dram_tensor("xh", [128, XCOLS], f16, kind="ExternalInput").ap()
    xl = nc.dram_tensor("xl", [128, XCOLS], f8e4, kind="ExternalInput").ap()
    ws = nc.dram_tensor("ws", [128, KT * 128], f16, kind="ExternalInput").ap()
    wsb = nc.dram_tensor("wsb", [128, KT * E], f16, kind="ExternalInput").ap()
    ident = nc.dram_tensor("ident", [E, E], f32, kind="ExternalInput").ap()
    out_w = nc.dram_tensor("out_w", [128, NTT * TOP_K], f32, kind="ExternalOutput").ap()
    out_i = nc.dram_tensor("out_i", [128, NTT * 8], i32, kind="ExternalOutput").ap()

    # panel -> start column (in tokens) and h-tile col offsets in the stream
    p_tok0 = [0, 1024, 1536]
    p_cols0 = [0, KT * 1024, KT * 1536]

    with tile.TileContext(nc) as tc:
        with (
            tc.tile_pool(name="persist", bufs=1) as persist,
            tc.tile_pool(name="work", bufs=4) as work,
            tc.tile_pool(name="psA", bufs=4, space="PSUM") as psA_pool,
            tc.tile_pool(name="psB", bufs=2, space="PSUM") as psB_pool,
            tc.tile_pool(name="psT", bufs=2, space="PSUM") as psT_pool,
        ):
            # ---------- input DMAs ----------
            # first two triggers ride the ACT queue (alive before Sync
            # finishes its semaphore prologue); the bulk stays on Sync
            _trig = {"n": 0}

            def trig_engine():
                _trig["n"] += 1
                return nc.scalar if _trig["n"] <= 2 else nc.sync

            ws_t = persist.tile([128, KT * 128], f16, tag="ws")
            trig_engine().dma_start(out=ws_t[:, 0:256], in_=ws[:, 0:256])

            xh_at = {}  # (p, a) -> (tile, col offset in tile, tokens)
            xl_at = {}

            def emit_x_chunks(p, chunks, src, at, dt_, kind):
                ptoks = PANELS[p]
                a0 = 0
                for c, sz in enumerate(chunks):
                    cols = sz * ptoks
                    off = p_cols0[p] + a0 * ptoks
                    t = persist.tile([128, cols], dt_, tag=f"{kind}{p}_{c}")
                    trig_engine().dma_start(out=t, in_=src[:, off : off + cols])
                    for j in range(sz):
                        at[(p, a0 + j)] = (t, j * ptoks)
                    a0 += sz

            # head: ws[a0:a1], xh chunk0, rest of ws, ident, wsb
            emit_x_chunks(0, CH_A[0][:1], xh, xh_at, f16, "xh")
            trig_engine().dma_start(out=ws_t[:, 256:], in_=ws[:, 256:])
            id_t = persist.tile([E, E], f32, tag="ident")
            trig_engine().dma_start(out=id_t, in_=ident)
            wsb_t = persist.tile([128, KT * E], f16, tag="wsb")
            trig_engine().dma_start(out=wsb_t, in_=wsb)

            def emit_rest(p, done_a, chunks, src, at, dt_, kind):
                # continue chunk emission after the first done_a h-tiles
                ptoks = PANELS[p]
                a0 = sum(chunks[:done_a])
                for c, sz in enumerate(chunks[done_a:]):
                    cols = sz * ptoks
                    off = p_cols0[p] + a0 * ptoks
                    t = persist.tile([128, cols], dt_, tag=f"{kind}{p}_{done_a + c}")
                    trig_engine().dma_start(out=t, in_=src[:, off : off + cols])
                    for j in range(sz):
                        at[(p, a0 + j)] = (t, j * ptoks)
                    a0 += sz

            emit_rest(0, 1, CH_A[0], xh, xh_at, f16, "xh")
            emit_x_chunks(0, CH_B[0], xl, xl_at, f8e4, "xl")
            for p in range(1, len(PANELS)):
                emit_x_chunks(p, CH_A[p], xh, xh_at, f16, "xh")
                emit_x_chunks(p, CH_B[p], xl, xl_at, f8e4, "xl")

            # ---------- PE warmup ----------
            junk = persist.tile([128, 512], f16, tag="junk")
            nc.vector.memset(junk, 1.0)
            ps_warm = psA_pool.tile([128, 512], f32, tag="psA")
            for _ in range(N_WARM):
                nc.tensor.matmul(
                    ps_warm, junk[:, 0:128], junk, start=True, stop=True
                )
            # absorb the ws / ident DMA semaphores on the PE
            nc.tensor.matmul(ps_warm, ws_t[:, 0:128], junk, start=True, stop=True)
            ps_warm2 = psT_pool.tile([128, E], f32, tag="ps_t")
            nc.tensor.transpose(ps_warm2[0:E, :], id_t, id_t)

            stage_w = persist.tile([128, NTT * TOP_K], f32, tag="stage_w")
            stage_i = persist.tile([128, NTT * 8], u32, tag="stage_i")

            # ---------- panels ----------
            psA = {}   # (p, b) -> [128, 512] tile
            psB = {}   # p -> [128, 512] tile (blocks packed in partition halves)
            lt = {}    # (p, b) -> [64, 512] sbuf logits.T

            pending_tiles = []  # epilogue tile closures from the previous panel

            def blocks(p):
                return PANELS[p] // 512

            def emit_passA_step(p, a):
                th, joff = xh_at[(p, a)]
                st = ws_t[:, a * 128 : (a + 1) * 128]
                for b in range(blocks(p)):
                    sl = slice(joff + b * 512, joff + (b + 1) * 512)
                    nc.tensor.matmul(
                        psA[(p, b)], st, th[:, sl],
                        start=(a == 0), stop=(a == KT - 1),
                    )

            def emit_passB_step(p, a):
                tl, joff = xl_at[(p, a)]
                st = wsb_t[:, a * E : (a + 1) * E]
                for b in range(blocks(p)):
                    sl = slice(joff + b * 512, joff + (b + 1) * 512)
                    nc.tensor.matmul(
                        psB_slice(p, b), st, tl[:, sl],
                        start=(a == 0), stop=(a == KT - 1),
                    )

            def emit_combine(p, b):
                # every psum term carries a 2^11 scale; combine is plain adds
                # (the 2^-11 is applied later inside the exp's scale arg)
                c1 = work.tile([64, 512], f32, tag="c1")
                nc.scalar.activation(
                    out=c1, in_=psB_slice(p, b),
                    func=mybir.ActivationFunctionType.Copy, scale=1.0,
                )
                u = work.tile([64, 512], f32, tag="u")
                nc.vector.tensor_add(u, c1, psA[(p, b)][64:128, :])
                ltb = work.tile([64, 512], f32, tag="lt", bufs=4)
                nc.vector.tensor_add(ltb, u, psA[(p, b)][0:64, :])
                lt[(p, b)] = ltb

            def emit_topk_tile(p, b, tt):
                # token tile index within the core
                t = (p_tok0[p] // 128) + b * 4 + tt
                ltb = lt[(p, b)]
                cs = slice(tt * 128, (tt + 1) * 128)
                ps_t = psT_pool.tile([128, E], f32, tag="ps_t")
                nc.tensor.transpose(ps_t, ltb[:, cs], id_t)
                m8 = work.tile([128, 8], f32, tag="m8")
                nc.vector.max(out=m8, in_=ps_t)
                nc.vector.max_index(stage_i[:, t * 8 : (t + 1) * 8], m8, ps_t)
                expw = work.tile([128, TOP_K], f32, tag="expw")
                ssum = work.tile([128, 1], f32, tag="ssum")
                nc.scalar.activation(
                    out=expw, in_=m8[:, 0:TOP_K],
                    func=mybir.ActivationFunctionType.Exp,
                    scale=LSCALE, accum_out=ssum[:, 0:1],
                )
                rsum = work.tile([128, 1], f32, tag="rsum")
                nc.vector.reciprocal(rsum, ssum)
                nc.vector.tensor_scalar_mul(
                    stage_w[:, t * TOP_K : (t + 1) * TOP_K], expw, rsum[:, 0:1]
                )

            def emit_out_dma(p):
                # output DMAs for panel p's token tiles, on the ACT queue
                c0 = p_tok0[p] // 128
                nt = PANELS[p] // 128
                nc.scalar.dma_start(
                    out=out_w[:, c0 * TOP_K : (c0 + nt) * TOP_K],
                    in_=stage_w[:, c0 * TOP_K : (c0 + nt) * TOP_K],
                )
                nc.scalar.dma_start(
                    out=out_i[:, c0 * 8 : (c0 + nt) * 8],
                    in_=stage_i[:, c0 * 8 : (c0 + nt) * 8].bitcast(i32),
                )

            def drain_pending(n):
                for _ in range(n):
                    if pending_tiles:
                        pending_tiles.pop(0)()

            def psB_slice(p, b):
                return psB[p][b * 64 : (b + 1) * 64, :]

            for p in range(len(PANELS)):
                for b in range(blocks(p)):
                    psA[(p, b)] = psA_pool.tile([128, 512], f32, tag="psA", name=f"psA_{p}_{b}")
                psB[p] = psB_pool.tile([128, 512], f32, tag="psB", name=f"psB_{p}")

                for a in range(KT):
                    emit_passA_step(p, a)
                    if p == 0 and 1 <= a <= 13:
                        # early phase is DMA-bound: keep the PE clock warm;
                        # extra spins at chunk boundaries bridge the longer
                        # completion waits so the clock never demotes
                        n_j = 3 if a in (1, 5, 10) else 1
                        for _ in range(n_j):
                            nc.tensor.matmul(
                                ps_warm, junk[:, 0:128], junk, start=True, stop=True
                            )
                for a in range(KT):
                    emit_passB_step(p, a)
                    if a % 3 == 2:
                        drain_pending(1)

                for b in range(blocks(p)):
                    emit_combine(p, b)
                for b in range(blocks(p)):
                    for tt in range(4):
                        pending_tiles.append(
                            (lambda p=p, b=b, tt=tt: emit_topk_tile(p, b, tt))
                        )
                if p > 0:
                    prev = p - 1
                    pending_tiles.append(lambda prev=prev: emit_out_dma(prev))

            drain_pending(len(pending_tiles))
            emit_out_dma(len(PANELS) - 1)

    nc.compile()
    return nc


def _get_nc():
    if "nc" not in _CACHE:
        _CACHE["nc"] = _build()
    return _CACHE["nc"]


def kernel(hidden_states: np.ndarray, weight: np.ndarray, **_run_kwargs):
    x = np.ascontiguousarray(hidden_states, dtype=np.float32).reshape(T_FULL, H)
    w = np.ascontiguousarray(weight, dtype=np.float32)

    # fused weight stationary: [wh_a | wl_a * 2^11] per h-tile
    wh = w.astype(np.float16)
    wl = ((w - wh.astype(np.float32)) * 2048.0).astype(np.float16)
    whs = (wh.astype(np.float32) * 2048.0).astype(np.float16)  # exact pow2 scale
    ws = np.zeros((128, KT * 128), dtype=np.float16)
    wsb = np.zeros((128, KT * E), dtype=np.float16)
    for a in range(KT):
        ws[:, a * 128 : a * 128 + E] = whs[:, a * 128 : (a + 1) * 128].T
        ws[:, a * 128 + E : (a + 1) * 128] = wl[:, a * 128 : (a + 1) * 128].T
        wsb[:, a * E : (a + 1) * E] = wh[:, a * 128 : (a + 1) * 128].T
    ident = np.eye(E, dtype=np.float32)

    p_tok0 = [0, 1024, 1536]

    def pack_stream(xT, dt_):
        # [H, T_CORE] -> [128, KT*T_CORE] in stream order (panel, h-tile)
        out = np.empty((128, KT * T_CORE), dtype=dt_)
        col = 0
        for p, ptoks in enumerate(PANELS):
            t0 = p_tok0[p]
            blk = xT[:, t0 : t0 + ptoks].reshape(KT, 128, ptoks)
            out[:, col : col + KT * ptoks] = (
                blk.transpose(1, 0, 2).reshape(128, KT * ptoks)
            )
            col += KT * ptoks
        return out

    in_maps = []
    for c in range(N_CORES):
        shard = x[c * T_CORE : (c + 1) * T_CORE, :]
        xT = np.ascontiguousarray(shard.T)          # [H, T_CORE] f32
        xh16 = xT.astype(np.float16)
        xl8 = ((xT - xh16.astype(np.float32)) * 2048.0).astype(e4m3)
        in_maps.append(
            {
                "xh": pack_stream(xh16, np.float16),
                "xl": pack_stream(xl8, e4m3),
                "ws": ws,
                "wsb": wsb,
                "ident": ident,
            }
        )

    nc = _get_nc()
    res = run_bass_kernel_spmd(
        nc, in_maps, core_ids=list(range(N_CORES)), **_run_kwargs
    )

    idx_parts = []
    w_parts = []
    for c in range(N_CORES):
        r = res.results[c]
        si = r["out_i"].reshape(128, NTT, 8).transpose(1, 0, 2)[:, :, :TOP_K]
        sw = r["out_w"].reshape(128, NTT, TOP_K).transpose(1, 0, 2)
        idx_parts.append(si.reshape(T_CORE, TOP_K).astype(np.int32, copy=False))
        w_parts.append(sw.reshape(T_CORE, TOP_K))

    topk_idx = np.concatenate(idx_parts, axis=0)
    topk_weight = np.concatenate(w_parts, axis=0)
    if "trace" in _run_kwargs:
        return (topk_idx, topk_weight), res
    return topk_idx, topk_weight


# revision 37
# speedup vs baseline: 1.0731x; 1.0731x over previous
"""MoE gate (top-6 routing) Trainium2 Bass kernel.

Problem: hidden_states [4, 4096, 2048] f32, gate weight [64, 2048] f32.
  logits = x @ W.T -> [16384, 64]; topk_weight, topk_idx = top_k(logits, 6);
  topk_weight = softmax(topk_weight)  (the extra normalization is a no-op).

Sharding: data-parallel over tokens, 2048 tokens/core, weight replicated.

Precision scheme (fp32-accurate top-6 at 3 bytes/element of HBM traffic,
vs 4 B for the previous two-fp16 scheme):
    xh = fp16(x)                      2 B   (moving, pass A)
    xl = fp8_e4m3((x - xh) * 2^11)    1 B   (moving, pass B)
    ws = [fp16(w)*2^11 | fp16((w - fp16(w)) * 2^11)]   (pass A stationary)
    wsb = fp16(w)                                      (pass B stationary)
    2^11 * logits = xh@ws_hi + xh@ws_lo + xl@wsb
All three PSUM terms carry the same 2^11 scale (powers of two are exact
in fp16), so the combine is plain adds and the 2^-11 lands for free in
the exp's scale argument; top-8/index ops are scale-invariant.
Verified on the actual test inputs: top-6 indices match the fp32 jax
reference on all 16384 tokens, robust to 1e-6 logit noise (hardware
accumulation order); weight max abs err ~4e-6.

Measured facts this design leans on (from hw probes/traces):
  - [LDWEIGHTS + 512-col fp16 matmul] issues every ~217 ns at full clock;
    ldweights hides under the previous matmul's stream.
  - fp8e4 moving data streams 2 cols/cycle (~110 ns per 512-col matmul),
    and fp16-stationary x fp8e4-moving is exact on hardware.
  - The PE clock ramps 0.65 -> 1.2 -> 2.4 GHz only under sustained busy;
    idle gaps reset it. Hence the early warmup spin + junk-fill matmuls
    while the first panel is DMA-bound.
  - Aggregate input-stream rate is ~360-420 GB/s across 16 DMA engines;
    packets are one per partition-row and LARGER rows stream faster
    (8 KB: ~26.5 B/ns vs 2 KB: ~14), so chunks are coarse where the PE
    has slack (panel 0) and finer where the PE runs tight (panels 1/2,
    whose pass B pays an exposed ldweights per matmul). The kernel is
    DMA-bound in steady state (~12.9 MiB at 3 B/element).

Kernel structure per core (2048 tokens, 16 contraction h-tiles of 128):
  - pass A: one fused [128, 128] fp16 stationary per h-tile produces
    xh@wh*2^11 (psum rows 0:64) and xh@wl*2^11 (rows 64:128) in one pass.
  - pass B: xl (fp8e4) against the 64-col wsb stationary; two 512-token
    blocks pack into partition halves of one PSUM bank.
  - tokens in 3 panels [1024, 512, 512]; h-tile-outer within each panel
    (long moving streams, stationary loaded once per h-tile per panel);
    panel epilogues drain between the next panel's passB steps; the
    small last panel keeps the serial tail short.
  - combine lt = psA_top + psA_bot + psB (ACT psum->sbuf copy, 2 DVE
    adds), PE-transpose per 128-token tile, DVE max8/max_index from
    PSUM, ACT exp(scale=2^-11) with accumulated sum, DVE reciprocal +
    scale into staging; per-panel output DMAs on the ACT queue.
"""

import numpy as np
import ml_dtypes

import concourse.mybir as mybir
import concourse.tile as tile
from concourse import bacc
from concourse.bass_utils import run_bass_kernel_spmd

f32 = mybir.dt.float32
f16 = mybir.dt.float16
f8e4 = mybir.dt.float8e4
u32 = mybir.dt.uint32
i32 = mybir.dt.int32
e4m3 = ml_dtypes.float8_e4m3

N_CORES = 8
B, S, H = 4, 4096, 2048
E = 64
TOP_K = 6
T_FULL = B * S
T_CORE = T_FULL // N_CORES   # 2048
KT = H // 128                # 16 h-tiles
PANELS = [1024, 512, 512]    # tokens per panel
NTT = T_CORE // 128          # 16 token tiles -> stage columns
LSCALE = float(2.0 ** -11)
N_WARM = 5

# chunk sizes (in h-tiles) for the x stream DMAs, per panel
CH_A = {0: [2, 4, 5, 5], 1: [5, 6, 5], 2: [5, 6, 5]}  # xh chunks
CH_B = {0: [8, 8], 1: [8, 8], 2: [8, 4, 4]}           # xl chunks

_CACHE = {}


def _build():
    nc = bacc.Bacc("TRN2", target_bir_lowering=False, debug=False)

    XCOLS = KT * T_CORE  # 32768 stream columns
    xh = nc.dram_tensor("xh", [128, XCOLS], f16, kind="ExternalInput").ap()
    xl = nc.dram_tensor("xl", [128, XCOLS], f8e4, kind="ExternalInput").ap()
    ws = nc.dram_tensor("ws", [128, KT * 128], f16, kind="ExternalInput").ap()
    wsb = nc.dram_tensor("wsb", [128, KT * E], f16, kind="ExternalInput").ap()
    ident = nc.dram_tensor("ident", [E, E], f32, kind="ExternalInput").ap()
    out_w = nc.dram_tensor("out_w", [128, NTT * TOP_K], f32, kind="ExternalOutput").ap()
    out_i = nc.dram_tensor("out_i", [128, NTT * 8], i32, kind="ExternalOutput").ap()

    # panel -> start column (in tokens) and h-tile col offsets in the stream
    p_tok0 = [0, 1024, 1536]
    p_cols0 = [0, KT * 1024, KT * 1536]

    with tile.TileContext(nc) as tc:
        with (
            tc.tile_pool(name="persist", bufs=1) as persist,
            tc.tile_pool(name="work", bufs=4) as work,
            tc.tile_pool(name="psA", bufs=4, space="PSUM") as psA_pool,
            tc.tile_pool(name="psB", bufs=1, space="PSUM") as psB_pool,
            tc.tile_pool(name="psT", bufs=3, space="PSUM") as psT_pool,
        ):
            # ---------- input DMAs ----------
            # first two triggers ride the ACT queue (alive before Sync
            # finishes its semaphore prologue); the bulk stays on Sync
            _trig = {"n": 0}

            def trig_engine():
                _trig["n"] += 1
                return nc.scalar if _trig["n"] <= 2 else nc.sync

            ws_t = persist.tile([128, KT * 128], f16, tag="ws")
            trig_engine().dma_start(out=ws_t[:, 0:256], in_=ws[:, 0:256])

            xh_at = {}  # (p, a) -> (tile, col offset in tile, tokens)
            xl_at = {}

            def emit_x_chunks(p, chunks, src, at, dt_, kind):
                ptoks = PANELS[p]
                a0 = 0
                for c, sz in enumerate(chunks):
                    cols = sz * ptoks
                    off = p_cols0[p] + a0 * ptoks
                    t = persist.tile([128, cols], dt_, tag=f"{kind}{p}_{c}")
                    trig_engine().dma_start(out=t, in_=src[:, off : off + cols])
                    for j in range(sz):
                        at[(p, a0 + j)] = (t, j * ptoks)
                    a0 += sz

            # head: ws[a0:a1], xh chunk0, rest of ws, ident, wsb
            emit_x_chunks(0, CH_A[0][:1], xh, xh_at, f16, "xh")
            trig_engine().dma_start(out=ws_t[:, 256:], in_=ws[:, 256:])
            id_t = persist.tile([E, E], f32, tag="ident")
            trig_engine().dma_start(out=id_t, in_=ident)
            wsb_t = persist.tile([128, KT * E], f16, tag="wsb")
            trig_engine().dma_start(out=wsb_t, in_=wsb)

            def emit_rest(p, done_a, chunks, src, at, dt_, kind):
                # continue chunk emission after the first done_a h-tiles
                ptoks = PANELS[p]
                a0 = sum(chunks[:done_a])
                for c, sz in enumerate(chunks[done_a:]):
                    cols = sz * ptoks
                    off = p_cols0[p] + a0 * ptoks
                    t = persist.tile([128, cols], dt_, tag=f"{kind}{p}_{done_a + c}")
                    trig_engine().dma_start(out=t, in_=src[:, off : off + cols])
                    for j in range(sz):
                        at[(p, a0 + j)] = (t, j * ptoks)
                    a0 += sz

            emit_rest(0, 1, CH_A[0], xh, xh_at, f16, "xh")
            emit_x_chunks(0, CH_B[0], xl, xl_at, f8e4, "xl")
            for p in range(1, len(PANELS)):
                emit_x_chunks(p, CH_A[p], xh, xh_at, f16, "xh")
                emit_x_chunks(p, CH_B[p], xl, xl_at, f8e4, "xl")

            # ---------- PE warmup ----------
            junk = persist.tile([128, 512], f16, tag="junk")
            nc.vector.memset(junk, 1.0)
            ps_warm = psA_pool.tile([128, 512], f32, tag="psA")
            for _ in range(N_WARM):
                nc.tensor.matmul(
                    ps_warm, junk[:, 0:128], junk, start=True, stop=True
                )
            # absorb the ws / ident DMA semaphores on the PE
            nc.tensor.matmul(ps_warm, ws_t[:, 0:128], junk, start=True, stop=True)
            ps_warm2 = psT_pool.tile([128, E], f32, tag="ps_t")
            nc.tensor.transpose(ps_warm2[0:E, :], id_t, id_t)

            stage_w = persist.tile([128, NTT * TOP_K], f32, tag="stage_w")
            stage_i = persist.tile([128, NTT * 8], u32, tag="stage_i")

            # ---------- panels ----------
            psA = {}   # (p, b) -> [128, 512] tile
            psB = {}   # p -> [128, 512] tile (blocks packed in partition halves)
            lt = {}    # (p, b) -> [64, 512] sbuf logits.T

            pending_tiles = []  # epilogue tile closures from the previous panel

            def blocks(p):
                return PANELS[p] // 512

            def emit_passA_step(p, a):
                th, joff = xh_at[(p, a)]
                st = ws_t[:, a * 128 : (a + 1) * 128]
                for b in range(blocks(p)):
                    sl = slice(joff + b * 512, joff + (b + 1) * 512)
                    nc.tensor.matmul(
                        psA[(p, b)], st, th[:, sl],
                        start=(a == 0), stop=(a == KT - 1),
                    )

            def emit_passB_step(p, a):
                tl, joff = xl_at[(p, a)]
                st = wsb_t[:, a * E : (a + 1) * E]
                for b in range(blocks(p)):
                    sl = slice(joff + b * 512, joff + (b + 1) * 512)
                    nc.tensor.matmul(
                        psB_slice(p, b), st, tl[:, sl],
                        start=(a == 0), stop=(a == KT - 1),
                    )

            def emit_combine(p, b):
                # every psum term carries a 2^11 scale; combine is plain adds
                # (the 2^-11 is applied later inside the exp's scale arg)
                c1 = work.tile([64, 512], f32, tag="c1")
                nc.scalar.activation(
                    out=c1, in_=psB_slice(p, b),
                    func=mybir.ActivationFunctionType.Copy, scale=1.0,
                )
                u = work.tile([64, 512], f32, tag="u")
                nc.vector.tensor_add(u, c1, psA[(p, b)][64:128, :])
                ltb = work.tile([64, 512], f32, tag="lt", bufs=4)
                nc.vector.tensor_add(ltb, u, psA[(p, b)][0:64, :])
                lt[(p, b)] = ltb

            def emit_topk_tile(p, b, tt):
                # token tile index within the core
                t = (p_tok0[p] // 128) + b * 4 + tt
                ltb = lt[(p, b)]
                cs = slice(tt * 128, (tt + 1) * 128)
                ps_t = psT_pool.tile([128, E], f32, tag="ps_t")
                nc.tensor.transpose(ps_t, ltb[:, cs], id_t)
                m8 = work.tile([128, 8], f32, tag="m8")
                nc.vector.max(out=m8, in_=ps_t)
                nc.vector.max_index(stage_i[:, t * 8 : (t + 1) * 8], m8, ps_t)
                expw = work.tile([128, TOP_K], f32, tag="expw")
                ssum = work.tile([128, 1], f32, tag="ssum")
                nc.scalar.activation(
                    out=expw, in_=m8[:, 0:TOP_K],
                    func=mybir.ActivationFunctionType.Exp,
                    scale=LSCALE, accum_out=ssum[:, 0:1],
                )
                rsum = work.tile([128, 1], f32, tag="rsum")
                nc.vector.reciprocal(rsum, ssum)
                nc.vector.tensor_scalar_mul(
                    stage_w[:, t * TOP_K : (t + 1) * TOP_K], expw, rsum[:, 0:1]
                )

            def emit_out_dma(p):
                # output DMAs for panel p's token tiles, on the ACT queue
                c0 = p_tok0[p] // 128
                nt = PANELS[p] // 128
                nc.scalar.dma_start(
                    out=out_w[:, c0 * TOP_K : (c0 + nt) * TOP_K],
                    in_=stage_w[:, c0 * TOP_K : (c0 + nt) * TOP_K],
                )
                nc.scalar.dma_start(
                    out=out_i[:, c0 * 8 : (c0 + nt) * 8],
                    in_=stage_i[:, c0 * 8 : (c0 + nt) * 8].bitcast(i32),
                )

            def drain_pending(n):
                for _ in range(n):
                    if pending_tiles:
                        pending_tiles.pop(0)()

            def psB_slice(p, b):
                return psB[p][b * 64 : (b + 1) * 64, :]

            for p in range(len(PANELS)):
                for b in range(blocks(p)):
                    psA[(p, b)] = psA_pool.tile([128, 512], f32, tag="psA", name=f"psA_{p}_{b}")
                psB[p] = psB_pool.tile([128, 512], f32, tag="psB", name=f"psB_{p}")

                for a in range(KT):
                    emit_passA_step(p, a)
                    if p == 0 and 1 <= a <= 13:
                        # early phase is DMA-bound: keep the PE clock warm;
                        # extra spins at chunk boundaries bridge the longer
                        # completion waits so the clock never demotes
                        n_j = 3 if a in (1, 5, 10) else 1
                        for _ in range(n_j):
                            nc.tensor.matmul(
                                ps_warm, junk[:, 0:128], junk, start=True, stop=True
                            )
                for a in range(KT):
                    emit_passB_step(p, a)
                    if a % 3 == 2:
                        drain_pending(1)

                for b in range(blocks(p)):
                    emit_combine(p, b)
                for b in range(blocks(p)):
                    for tt in range(4):
                        pending_tiles.append(
                            (lambda p=p, b=b, tt=tt: emit_topk_tile(p, b, tt))
                        )
                if p > 0:
                    prev = p - 1
                    pending_tiles.append(lambda prev=prev: emit_out_dma(prev))

            drain_pending(len(pending_tiles))
            emit_out_dma(len(PANELS) - 1)

    nc.compile()
    return nc


def _get_nc():
    if "nc" not in _CACHE:
        _CACHE["nc"] = _build()
    return _CACHE["nc"]


def kernel(hidden_states: np.ndarray, weight: np.ndarray, **_run_kwargs):
    x = np.ascontiguousarray(hidden_states, dtype=np.float32).reshape(T_FULL, H)
    w = np.ascontiguousarray(weight, dtype=np.float32)

    # fused weight stationary: [wh_a | wl_a * 2^11] per h-tile
    wh = w.astype(np.float16)
    wl = ((w - wh.astype(np.float32)) * 2048.0).astype(np.float16)
    whs = (wh.astype(np.float32) * 2048.0).astype(np.float16)  # exact pow2 scale
    ws = np.zeros((128, KT * 128), dtype=np.float16)
    wsb = np.zeros((128, KT * E), dtype=np.float16)
    for a in range(KT):
        ws[:, a * 128 : a * 128 + E] = whs[:, a * 128 : (a + 1) * 128].T
        ws[:, a * 128 + E : (a + 1) * 128] = wl[:, a * 128 : (a + 1) * 128].T
        wsb[:, a * E : (a + 1) * E] = wh[:, a * 128 : (a + 1) * 128].T
    ident = np.eye(E, dtype=np.float32)

    p_tok0 = [0, 1024, 1536]

    def pack_stream(xT, dt_):
        # [H, T_CORE] -> [128, KT*T_CORE] in stream order (panel, h-tile)
        out = np.empty((128, KT * T_CORE), dtype=dt_)
        col = 0
        for p, ptoks in enumerate(PANELS):
            t0 = p_tok0[p]
            blk = xT[:, t0 : t0 + ptoks].reshape(KT, 128, ptoks)
            out[:, col : col + KT * ptoks] = (
                blk.transpose(1, 0, 2).reshape(128, KT * ptoks)
            )
            col += KT * ptoks
        return out

    in_maps = []
    for c in range(N_CORES):
        shard = x[c * T_CORE : (c + 1) * T_CORE, :]
        xT = np.ascontiguousarray(shard.T)          # [H, T_CORE] f32
        xh16 = xT.astype(np.float16)
        xl8 = ((xT - xh16.astype(np.float32)) * 2048.0).astype(e4m3)
        in_maps.append(
            {
                "xh": pack_stream(xh16, np.float16),
                "xl": pack_stream(xl8, e4m3),
                "ws": ws,
                "wsb": wsb,
                "ident": ident,
            }
        )

    nc = _get_nc()
    res = run_bass_kernel_spmd(
        nc, in_maps, core_ids=list(range(N_CORES)), **_run_kwargs
    )

    idx_parts = []
    w_parts = []
    for c in range(N_CORES):
        r = res.results[c]
        si = r["out_i"].reshape(128, NTT, 8).transpose(1, 0, 2)[:, :, :TOP_K]
        sw = r["out_w"].reshape(128, NTT, TOP_K).transpose(1, 0, 2)
        idx_parts.append(si.reshape(T_CORE, TOP_K).astype(np.int32, copy=False))
        w_parts.append(sw.reshape(T_CORE, TOP_K))

    topk_idx = np.concatenate(idx_parts, axis=0)
    topk_weight = np.concatenate(w_parts, axis=0)
    if "trace" in _run_kwargs:
        return (topk_idx, topk_weight), res
    return topk_idx, topk_weight


# revision 38
# speedup vs baseline: 1.1232x; 1.0467x over previous
"""MoE gate (top-6 routing) Trainium2 Bass kernel.

Problem: hidden_states [4, 4096, 2048] f32, gate weight [64, 2048] f32.
  logits = x @ W.T -> [16384, 64]; topk_weight, topk_idx = top_k(logits, 6);
  topk_weight = softmax(topk_weight)  (the extra normalization is a no-op).

Sharding: data-parallel over tokens, 2048 tokens/core, weight replicated.

Precision scheme (fp32-accurate top-6 at 3 bytes/element of HBM traffic,
vs 4 B for the previous two-fp16 scheme):
    xh = fp16(x)                      2 B   (moving, pass A)
    xl = fp8_e4m3((x - xh) * 2^11)    1 B   (moving, pass B)
    ws = [fp16(w)*2^11 | fp16((w - fp16(w)) * 2^11)]   (pass A stationary)
    wsb = fp16(w)                                      (pass B stationary)
    2^11 * logits = xh@ws_hi + xh@ws_lo + xl@wsb
All three PSUM terms carry the same 2^11 scale (powers of two are exact
in fp16), so the combine is plain adds and the 2^-11 lands for free in
the exp's scale argument; top-8/index ops are scale-invariant.
Verified on the actual test inputs: top-6 indices match the fp32 jax
reference on all 16384 tokens, robust to 1e-6 logit noise (hardware
accumulation order); weight max abs err ~4e-6.

Measured facts this design leans on (from hw probes/traces):
  - [LDWEIGHTS + 512-col fp16 matmul] issues every ~217 ns at full clock;
    ldweights hides under the previous matmul's stream.
  - fp8e4 moving data streams 2 cols/cycle (~110 ns per 512-col matmul),
    and fp16-stationary x fp8e4-moving is exact on hardware.
  - The PE clock ramps 0.65 -> 1.2 -> 2.4 GHz only under sustained busy;
    idle gaps reset it. Hence the early warmup spin + junk-fill matmuls
    while the first panel is DMA-bound.
  - Aggregate input-stream rate is ~360-420 GB/s across 16 DMA engines;
    packets are one per partition-row and LARGER rows stream faster
    (8 KB: ~26.5 B/ns vs 2 KB: ~14), so chunks are coarse where the PE
    has slack (panel 0) and finer where the PE runs tight (panels 1/2,
    whose pass B pays an exposed ldweights per matmul). The kernel is
    DMA-bound in steady state (~12.9 MiB at 3 B/element).

Kernel structure per core (2048 tokens, 16 contraction h-tiles of 128):
  - pass A: one fused [128, 128] fp16 stationary per h-tile produces
    xh@wh*2^11 (psum rows 0:64) and xh@wl*2^11 (rows 64:128) in one pass.
  - pass B: xl (fp8e4) against the 64-col wsb stationary; two 512-token
    blocks pack into partition halves of one PSUM bank.
  - tokens in 3 panels [1024, 512, 512]; h-tile-outer within each panel
    (long moving streams, stationary loaded once per h-tile per panel);
    panel epilogues drain between the next panel's passB steps; the
    small last panel keeps the serial tail short.
  - combine lt = psA_top + psA_bot + psB (ACT psum->sbuf copy, 2 DVE
    adds), PE-transpose per 128-token tile, DVE max8/max_index from
    PSUM, ACT exp(scale=2^-11) with accumulated sum, DVE reciprocal +
    scale into staging; per-panel output DMAs on the ACT queue.
"""

import numpy as np
import ml_dtypes

import concourse.mybir as mybir
import concourse.tile as tile
from concourse import bacc
from concourse.bass_utils import run_bass_kernel_spmd

f32 = mybir.dt.float32
f16 = mybir.dt.float16
f8e4 = mybir.dt.float8e4
u32 = mybir.dt.uint32
i32 = mybir.dt.int32
e4m3 = ml_dtypes.float8_e4m3

N_CORES = 8
B, S, H = 4, 4096, 2048
E = 64
TOP_K = 6
T_FULL = B * S
T_CORE = T_FULL // N_CORES   # 2048
KT = H // 128                # 16 h-tiles
PANELS = [1024, 512, 512]    # tokens per panel
NTT = T_CORE // 128          # 16 token tiles -> stage columns
LSCALE = float(2.0 ** -11)
N_WARM = 5

# chunk sizes (in h-tiles) for the x stream DMAs, per panel
CH_A = {0: [2, 4, 5, 5], 1: [5, 6, 5], 2: [5, 6, 5]}  # xh chunks
CH_B = {0: [8, 8], 1: [8, 8], 2: [8, 4, 2, 2]}        # xl chunks

_CACHE = {}


def _build():
    nc = bacc.Bacc("TRN2", target_bir_lowering=False, debug=False)

    XCOLS = KT * T_CORE  # 32768 stream columns
    xh = nc.dram_tensor("xh", [128, XCOLS], f16, kind="ExternalInput").ap()
    xl = nc.dram_tensor("xl", [128, XCOLS], f8e4, kind="ExternalInput").ap()
    ws = nc.dram_tensor("ws", [128, KT * 128], f16, kind="ExternalInput").ap()
    wsb = nc.dram_tensor("wsb", [128, KT * E], f16, kind="ExternalInput").ap()
    ident = nc.dram_tensor("ident", [E, E], f32, kind="ExternalInput").ap()
    out_w = nc.dram_tensor("out_w", [128, NTT * TOP_K], f32, kind="ExternalOutput").ap()
    out_i = nc.dram_tensor("out_i", [128, NTT * 8], i32, kind="ExternalOutput").ap()

    # panel -> start column (in tokens) and h-tile col offsets in the stream
    p_tok0 = [0, 1024, 1536]
    p_cols0 = [0, KT * 1024, KT * 1536]

    with tile.TileContext(nc) as tc:
        with (
            tc.tile_pool(name="persist", bufs=1) as persist,
            tc.tile_pool(name="work", bufs=4) as work,
            tc.tile_pool(name="psA", bufs=4, space="PSUM") as psA_pool,
            tc.tile_pool(name="psB", bufs=1, space="PSUM") as psB_pool,
            tc.tile_pool(name="psT", bufs=3, space="PSUM") as psT_pool,
        ):
            # ---------- input DMAs ----------
            # first two triggers ride the ACT queue (alive before Sync
            # finishes its semaphore prologue); the bulk stays on Sync
            _trig = {"n": 0}

            def trig_engine():
                _trig["n"] += 1
                return nc.scalar if _trig["n"] <= 2 else nc.sync

            ws_t = persist.tile([128, KT * 128], f16, tag="ws")
            trig_engine().dma_start(out=ws_t[:, 0:256], in_=ws[:, 0:256])

            xh_at = {}  # (p, a) -> (tile, col offset in tile, tokens)
            xl_at = {}

            def emit_x_chunks(p, chunks, src, at, dt_, kind):
                ptoks = PANELS[p]
                a0 = 0
                for c, sz in enumerate(chunks):
                    cols = sz * ptoks
                    off = p_cols0[p] + a0 * ptoks
                    t = persist.tile([128, cols], dt_, tag=f"{kind}{p}_{c}")
                    trig_engine().dma_start(out=t, in_=src[:, off : off + cols])
                    for j in range(sz):
                        at[(p, a0 + j)] = (t, j * ptoks)
                    a0 += sz

            # head: ws[a0:a1], xh chunk0, rest of ws, ident, wsb
            emit_x_chunks(0, CH_A[0][:1], xh, xh_at, f16, "xh")
            trig_engine().dma_start(out=ws_t[:, 256:], in_=ws[:, 256:])
            id_t = persist.tile([E, E], f32, tag="ident")
            trig_engine().dma_start(out=id_t, in_=ident)
            wsb_t = persist.tile([128, KT * E], f16, tag="wsb")
            trig_engine().dma_start(out=wsb_t, in_=wsb)

            def emit_rest(p, done_a, chunks, src, at, dt_, kind):
                # continue chunk emission after the first done_a h-tiles
                ptoks = PANELS[p]
                a0 = sum(chunks[:done_a])
                for c, sz in enumerate(chunks[done_a:]):
                    cols = sz * ptoks
                    off = p_cols0[p] + a0 * ptoks
                    t = persist.tile([128, cols], dt_, tag=f"{kind}{p}_{done_a + c}")
                    trig_engine().dma_start(out=t, in_=src[:, off : off + cols])
                    for j in range(sz):
                        at[(p, a0 + j)] = (t, j * ptoks)
                    a0 += sz

            emit_rest(0, 1, CH_A[0], xh, xh_at, f16, "xh")
            emit_x_chunks(0, CH_B[0], xl, xl_at, f8e4, "xl")
            for p in range(1, len(PANELS)):
                emit_x_chunks(p, CH_A[p], xh, xh_at, f16, "xh")
                emit_x_chunks(p, CH_B[p], xl, xl_at, f8e4, "xl")

            # ---------- PE warmup ----------
            junk = persist.tile([128, 512], f16, tag="junk")
            nc.vector.memset(junk, 1.0)
            ps_warm = psA_pool.tile([128, 512], f32, tag="psA")
            for _ in range(N_WARM):
                nc.tensor.matmul(
                    ps_warm, junk[:, 0:128], junk, start=True, stop=True
                )
            # absorb the ws / ident DMA semaphores on the PE
            nc.tensor.matmul(ps_warm, ws_t[:, 0:128], junk, start=True, stop=True)
            ps_warm2 = psT_pool.tile([128, E], f32, tag="ps_t")
            nc.tensor.transpose(ps_warm2[0:E, :], id_t, id_t)

            stage_w = persist.tile([128, NTT * TOP_K], f32, tag="stage_w")
            stage_i = persist.tile([128, NTT * 8], u32, tag="stage_i")

            # ---------- panels ----------
            psA = {}   # (p, b) -> [128, 512] tile
            psB = {}   # p -> [128, 512] tile (blocks packed in partition halves)
            lt = {}    # (p, b) -> [64, 512] sbuf logits.T

            pending_tiles = []  # epilogue tile closures from the previous panel

            def blocks(p):
                return PANELS[p] // 512

            def emit_passA_step(p, a):
                th, joff = xh_at[(p, a)]
                st = ws_t[:, a * 128 : (a + 1) * 128]
                for b in range(blocks(p)):
                    sl = slice(joff + b * 512, joff + (b + 1) * 512)
                    nc.tensor.matmul(
                        psA[(p, b)], st, th[:, sl],
                        start=(a == 0), stop=(a == KT - 1),
                    )

            def emit_passB_step(p, a):
                tl, joff = xl_at[(p, a)]
                st = wsb_t[:, a * E : (a + 1) * E]
                for b in range(blocks(p)):
                    sl = slice(joff + b * 512, joff + (b + 1) * 512)
                    nc.tensor.matmul(
                        psB_slice(p, b), st, tl[:, sl],
                        start=(a == 0), stop=(a == KT - 1),
                    )

            lt_ab = {}

            def emit_combine_ab(p, b):
                # psA stopped at end of pass A: fold its halves during pass B
                cab = work.tile([64, 512], f32, tag="c1")
                nc.scalar.activation(
                    out=cab, in_=psA[(p, b)][64:128, :],
                    func=mybir.ActivationFunctionType.Copy, scale=1.0,
                )
                ab = work.tile([64, 512], f32, tag="ltab", bufs=4)
                nc.vector.tensor_add(ab, cab, psA[(p, b)][0:64, :])
                lt_ab[(p, b)] = ab

            def emit_combine(p, b):
                # single DVE add on the post-passB critical path
                ltb = work.tile([64, 512], f32, tag="lt", bufs=4)
                nc.vector.tensor_add(ltb, lt_ab[(p, b)], psB_slice(p, b))
                lt[(p, b)] = ltb

            def emit_topk_tile(p, b, tt):
                # token tile index within the core
                t = (p_tok0[p] // 128) + b * 4 + tt
                ltb = lt[(p, b)]
                cs = slice(tt * 128, (tt + 1) * 128)
                ps_t = psT_pool.tile([128, E], f32, tag="ps_t")
                nc.tensor.transpose(ps_t, ltb[:, cs], id_t)
                m8 = work.tile([128, 8], f32, tag="m8")
                nc.vector.max(out=m8, in_=ps_t)
                nc.vector.max_index(stage_i[:, t * 8 : (t + 1) * 8], m8, ps_t)
                expw = work.tile([128, TOP_K], f32, tag="expw")
                ssum = work.tile([128, 1], f32, tag="ssum")
                nc.scalar.activation(
                    out=expw, in_=m8[:, 0:TOP_K],
                    func=mybir.ActivationFunctionType.Exp,
                    scale=LSCALE, accum_out=ssum[:, 0:1],
                )
                rsum = work.tile([128, 1], f32, tag="rsum")
                nc.vector.reciprocal(rsum, ssum)
                nc.vector.tensor_scalar_mul(
                    stage_w[:, t * TOP_K : (t + 1) * TOP_K], expw, rsum[:, 0:1]
                )

            def emit_out_dma(p):
                # output DMAs for panel p's token tiles, on the ACT queue
                c0 = p_tok0[p] // 128
                nt = PANELS[p] // 128
                nc.sync.dma_start(
                    out=out_w[:, c0 * TOP_K : (c0 + nt) * TOP_K],
                    in_=stage_w[:, c0 * TOP_K : (c0 + nt) * TOP_K],
                )
                nc.scalar.dma_start(
                    out=out_i[:, c0 * 8 : (c0 + nt) * 8],
                    in_=stage_i[:, c0 * 8 : (c0 + nt) * 8].bitcast(i32),
                )

            def drain_pending(n):
                for _ in range(n):
                    if pending_tiles:
                        pending_tiles.pop(0)()

            def psB_slice(p, b):
                return psB[p][b * 64 : (b + 1) * 64, :]

            for p in range(len(PANELS)):
                for b in range(blocks(p)):
                    psA[(p, b)] = psA_pool.tile([128, 512], f32, tag="psA", name=f"psA_{p}_{b}")
                psB[p] = psB_pool.tile([128, 512], f32, tag="psB", name=f"psB_{p}")

                for a in range(KT):
                    emit_passA_step(p, a)
                    if p == 0 and 1 <= a <= 13:
                        # early phase is DMA-bound: keep the PE clock warm;
                        # extra spins at chunk boundaries bridge the longer
                        # completion waits so the clock never demotes
                        n_j = 3 if a in (1, 5, 10) else 1
                        for _ in range(n_j):
                            nc.tensor.matmul(
                                ps_warm, junk[:, 0:128], junk, start=True, stop=True
                            )
                for a in range(KT):
                    emit_passB_step(p, a)
                    if a % 3 == 2:
                        drain_pending(1)
                    if a == 3:
                        for b in range(blocks(p)):
                            emit_combine_ab(p, b)

                for b in range(blocks(p)):
                    emit_combine(p, b)
                for b in range(blocks(p)):
                    for tt in range(4):
                        pending_tiles.append(
                            (lambda p=p, b=b, tt=tt: emit_topk_tile(p, b, tt))
                        )
                if p > 0:
                    prev = p - 1
                    pending_tiles.append(lambda prev=prev: emit_out_dma(prev))

            drain_pending(len(pending_tiles))
            emit_out_dma(len(PANELS) - 1)

    nc.compile()
    return nc


def _get_nc():
    if "nc" not in _CACHE:
        _CACHE["nc"] = _build()
    return _CACHE["nc"]


def kernel(hidden_states: np.ndarray, weight: np.ndarray, **_run_kwargs):
    x = np.ascontiguousarray(hidden_states, dtype=np.float32).reshape(T_FULL, H)
    w = np.ascontiguousarray(weight, dtype=np.float32)

    # fused weight stationary: [wh_a | wl_a * 2^11] per h-tile
    wh = w.astype(np.float16)
    wl = ((w - wh.astype(np.float32)) * 2048.0).astype(np.float16)
    whs = (wh.astype(np.float32) * 2048.0).astype(np.float16)  # exact pow2 scale
    ws = np.zeros((128, KT * 128), dtype=np.float16)
    wsb = np.zeros((128, KT * E), dtype=np.float16)
    for a in range(KT):
        ws[:, a * 128 : a * 128 + E] = whs[:, a * 128 : (a + 1) * 128].T
        ws[:, a * 128 + E : (a + 1) * 128] = wl[:, a * 128 : (a + 1) * 128].T
        wsb[:, a * E : (a + 1) * E] = wh[:, a * 128 : (a + 1) * 128].T
    ident = np.eye(E, dtype=np.float32)

    p_tok0 = [0, 1024, 1536]

    def pack_stream(xT, dt_):
        # [H, T_CORE] -> [128, KT*T_CORE] in stream order (panel, h-tile)
        out = np.empty((128, KT * T_CORE), dtype=dt_)
        col = 0
        for p, ptoks in enumerate(PANELS):
            t0 = p_tok0[p]
            blk = xT[:, t0 : t0 + ptoks].reshape(KT, 128, ptoks)
            out[:, col : col + KT * ptoks] = (
                blk.transpose(1, 0, 2).reshape(128, KT * ptoks)
            )
            col += KT * ptoks
        return out

    in_maps = []
    for c in range(N_CORES):
        shard = x[c * T_CORE : (c + 1) * T_CORE, :]
        xT = np.ascontiguousarray(shard.T)          # [H, T_CORE] f32
        xh16 = xT.astype(np.float16)
        xl8 = ((xT - xh16.astype(np.float32)) * 2048.0).astype(e4m3)
        in_maps.append(
            {
                "xh": pack_stream(xh16, np.float16),
                "xl": pack_stream(xl8, e4m3),
                "ws": ws,
                "wsb": wsb,
                "ident": ident,
            }
        )

    nc = _get_nc()
    res = run_bass_kernel_spmd(
        nc, in_maps, core_ids=list(range(N_CORES)), **_run_kwargs
    )

    idx_parts = []
    w_parts = []
    for c in range(N_CORES):
        r = res.results[c]
        si = r["out_i"].reshape(128, NTT, 8).transpose(1, 0, 2)[:, :, :TOP_K]
        sw = r["out_w"].reshape(128, NTT, TOP_K).transpose(1, 0, 2)
        idx_parts.append(si.reshape(T_CORE, TOP_K).astype(np.int32, copy=False))
        w_parts.append(sw.reshape(T_CORE, TOP_K))

    topk_idx = np.concatenate(idx_parts, axis=0)
    topk_weight = np.concatenate(w_parts, axis=0)
    if "trace" in _run_kwargs:
        return (topk_idx, topk_weight), res
    return topk_idx, topk_weight


# revision 39
# speedup vs baseline: 1.1885x; 1.0582x over previous
"""MoE gate (top-6 routing) Trainium2 Bass kernel.

Problem: hidden_states [4, 4096, 2048] f32, gate weight [64, 2048] f32.
  logits = x @ W.T -> [16384, 64]; topk_weight, topk_idx = top_k(logits, 6);
  topk_weight = softmax(topk_weight)  (the extra normalization is a no-op).

Sharding: data-parallel over tokens, 2048 tokens/core, weight replicated.

Precision scheme (fp32-accurate top-6 at 3 bytes/element of HBM traffic,
vs 4 B for the previous two-fp16 scheme):
    xh = fp16(x)                      2 B   (moving, pass A)
    xl = fp8_e4m3((x - xh) * 2^11)    1 B   (moving, pass B)
    ws = [fp16(w)*2^11 | fp16((w - fp16(w)) * 2^11)]   (pass A stationary)
    wsb = fp16(w)                                      (pass B stationary)
    2^11 * logits = xh@ws_hi + xh@ws_lo + xl@wsb
All three PSUM terms carry the same 2^11 scale (powers of two are exact
in fp16), so the combine is plain adds and the 2^-11 lands for free in
the exp's scale argument; top-8/index ops are scale-invariant.
Verified on the actual test inputs: top-6 indices match the fp32 jax
reference on all 16384 tokens, robust to 1e-6 logit noise (hardware
accumulation order); weight max abs err ~4e-6.

Measured facts this design leans on (from hw probes/traces):
  - [LDWEIGHTS + 512-col fp16 matmul] issues every ~217 ns at full clock;
    ldweights hides under the previous matmul's stream.
  - fp8e4 moving data streams 2 cols/cycle (~110 ns per 512-col matmul),
    and fp16-stationary x fp8e4-moving is exact on hardware.
  - The PE clock ramps 0.65 -> 1.2 -> 2.4 GHz only under sustained busy;
    idle gaps reset it. Hence the early warmup spin + junk-fill matmuls
    while the first panel is DMA-bound.
  - Aggregate input-stream rate is ~360-420 GB/s across 16 DMA engines;
    packets are one per partition-row and LARGER rows stream faster
    (8 KB: ~26.5 B/ns vs 2 KB: ~14), so chunks are coarse where the PE
    has slack (panel 0) and finer where the PE runs tight (panels 1/2,
    whose pass B pays an exposed ldweights per matmul). The kernel is
    DMA-bound in steady state (~12.9 MiB at 3 B/element).

Kernel structure per core (2048 tokens, 16 contraction h-tiles of 128):
  - pass A: one fused [128, 128] fp16 stationary per h-tile produces
    xh@wh*2^11 (psum rows 0:64) and xh@wl*2^11 (rows 64:128) in one pass.
  - pass B: xl (fp8e4) against the 64-col wsb stationary; two 512-token
    blocks pack into partition halves of one PSUM bank.
  - tokens in 3 panels [1024, 512, 512]; h-tile-outer within each panel
    (long moving streams, stationary loaded once per h-tile per panel);
    panel epilogues drain between the next panel's passB steps; the
    small last panel keeps the serial tail short.
  - combine lt = psA_top + psA_bot + psB (ACT psum->sbuf copy, 2 DVE
    adds), PE-transpose per 128-token tile, DVE max8/max_index from
    PSUM, ACT exp(scale=2^-11) with accumulated sum, DVE reciprocal +
    scale into staging; per-panel output DMAs on the ACT queue.
"""

import numpy as np
import ml_dtypes

import concourse.mybir as mybir
import concourse.tile as tile
from concourse import bacc
from concourse.bass_utils import run_bass_kernel_spmd

f32 = mybir.dt.float32
f16 = mybir.dt.float16
f8e4 = mybir.dt.float8e4
u32 = mybir.dt.uint32
i32 = mybir.dt.int32
e4m3 = ml_dtypes.float8_e4m3

N_CORES = 8
B, S, H = 4, 4096, 2048
E = 64
TOP_K = 6
T_FULL = B * S
T_CORE = T_FULL // N_CORES   # 2048
KT = H // 128                # 16 h-tiles
PANELS = [1024, 512, 512]    # tokens per panel
NTT = T_CORE // 128          # 16 token tiles -> stage columns
LSCALE = float(2.0 ** -11)
N_WARM = 5

# chunk sizes (in h-tiles) for the x stream DMAs, per panel
CH_A = {0: [2, 4, 5, 5], 1: [5, 6, 5], 2: [5, 6, 5]}  # xh chunks
CH_B = {0: [8, 8], 1: [8, 8], 2: [8, 4, 2, 2]}        # xl chunks

_CACHE = {}


def _build():
    nc = bacc.Bacc("TRN2", target_bir_lowering=False, debug=False)

    XCOLS = KT * T_CORE  # 32768 stream columns
    xh = nc.dram_tensor("xh", [128, XCOLS], f16, kind="ExternalInput").ap()
    xl = nc.dram_tensor("xl", [128, XCOLS], f8e4, kind="ExternalInput").ap()
    ws = nc.dram_tensor("ws", [128, KT * 128], f16, kind="ExternalInput").ap()
    wsb = nc.dram_tensor("wsb", [128, KT * E], f16, kind="ExternalInput").ap()
    ident = nc.dram_tensor("ident", [E, E], f32, kind="ExternalInput").ap()
    out_w = nc.dram_tensor("out_w", [128, NTT * TOP_K], f32, kind="ExternalOutput").ap()
    out_i = nc.dram_tensor("out_i", [128, NTT * 8], i32, kind="ExternalOutput").ap()

    # panel -> start column (in tokens) and h-tile col offsets in the stream
    p_tok0 = [0, 1024, 1536]
    p_cols0 = [0, KT * 1024, KT * 1536]

    with tile.TileContext(nc) as tc:
        with (
            tc.tile_pool(name="persist", bufs=1) as persist,
            tc.tile_pool(name="work", bufs=4) as work,
            tc.tile_pool(name="psA", bufs=4, space="PSUM") as psA_pool,
            tc.tile_pool(name="psB", bufs=1, space="PSUM") as psB_pool,
            tc.tile_pool(name="psT", bufs=3, space="PSUM") as psT_pool,
        ):
            # ---------- input DMAs ----------
            # first two triggers ride the ACT queue (alive before Sync
            # finishes its semaphore prologue); the bulk stays on Sync
            _trig = {"n": 0}

            def trig_engine():
                _trig["n"] += 1
                return nc.scalar if _trig["n"] <= 2 else nc.sync

            ws_t = persist.tile([128, KT * 128], f16, tag="ws")
            trig_engine().dma_start(out=ws_t[:, 0:256], in_=ws[:, 0:256])

            xh_at = {}  # (p, a) -> (tile, col offset in tile, tokens)
            xl_at = {}

            def emit_x_chunks(p, chunks, src, at, dt_, kind):
                ptoks = PANELS[p]
                a0 = 0
                for c, sz in enumerate(chunks):
                    cols = sz * ptoks
                    off = p_cols0[p] + a0 * ptoks
                    t = persist.tile([128, cols], dt_, tag=f"{kind}{p}_{c}")
                    trig_engine().dma_start(out=t, in_=src[:, off : off + cols])
                    for j in range(sz):
                        at[(p, a0 + j)] = (t, j * ptoks)
                    a0 += sz

            # head: ws[a0:a1], xh chunk0, rest of ws, ident, wsb
            emit_x_chunks(0, CH_A[0][:1], xh, xh_at, f16, "xh")
            trig_engine().dma_start(out=ws_t[:, 256:], in_=ws[:, 256:])
            id_t = persist.tile([E, E], f32, tag="ident")
            trig_engine().dma_start(out=id_t, in_=ident)
            wsb_t = persist.tile([128, KT * E], f16, tag="wsb")
            trig_engine().dma_start(out=wsb_t, in_=wsb)

            def emit_rest(p, done_a, chunks, src, at, dt_, kind):
                # continue chunk emission after the first done_a h-tiles
                ptoks = PANELS[p]
                a0 = sum(chunks[:done_a])
                for c, sz in enumerate(chunks[done_a:]):
                    cols = sz * ptoks
                    off = p_cols0[p] + a0 * ptoks
                    t = persist.tile([128, cols], dt_, tag=f"{kind}{p}_{done_a + c}")
                    trig_engine().dma_start(out=t, in_=src[:, off : off + cols])
                    for j in range(sz):
                        at[(p, a0 + j)] = (t, j * ptoks)
                    a0 += sz

            emit_rest(0, 1, CH_A[0], xh, xh_at, f16, "xh")
            emit_x_chunks(0, CH_B[0], xl, xl_at, f8e4, "xl")
            for p in range(1, len(PANELS)):
                emit_x_chunks(p, CH_A[p], xh, xh_at, f16, "xh")
                emit_x_chunks(p, CH_B[p], xl, xl_at, f8e4, "xl")

            # ---------- PE warmup ----------
            junk = persist.tile([128, 512], f16, tag="junk")
            nc.vector.memset(junk, 1.0)
            ps_warm = psA_pool.tile([128, 512], f32, tag="psA")
            for _ in range(N_WARM):
                nc.tensor.matmul(
                    ps_warm, junk[:, 0:128], junk, start=True, stop=True
                )
            # absorb the ws / ident DMA semaphores on the PE
            nc.tensor.matmul(ps_warm, ws_t[:, 0:128], junk, start=True, stop=True)
            ps_warm2 = psT_pool.tile([128, E], f32, tag="ps_t")
            nc.tensor.transpose(ps_warm2[0:E, :], id_t, id_t)

            stage_w = persist.tile([128, NTT * TOP_K], f32, tag="stage_w")
            stage_i = persist.tile([128, NTT * 8], u32, tag="stage_i")

            # ---------- panels ----------
            psA = {}   # (p, b) -> [128, 512] tile
            psB = {}   # p -> [128, 512] tile (blocks packed in partition halves)
            lt = {}    # (p, b) -> [64, 512] sbuf logits.T

            pending_tiles = []  # epilogue tile closures from the previous panel

            def blocks(p):
                return PANELS[p] // 512

            def emit_passA_step(p, a):
                th, joff = xh_at[(p, a)]
                st = ws_t[:, a * 128 : (a + 1) * 128]
                for b in range(blocks(p)):
                    sl = slice(joff + b * 512, joff + (b + 1) * 512)
                    nc.tensor.matmul(
                        psA[(p, b)], st, th[:, sl],
                        start=(a == 0), stop=(a == KT - 1),
                    )

            def emit_passB_step(p, a):
                tl, joff = xl_at[(p, a)]
                st = wsb_t[:, a * E : (a + 1) * E]
                if blocks(p) == 1:
                    # two 256-col matmuls per step so the next ldweights
                    # hides under the second (one 107ns fp8 matmul alone
                    # cannot cover a ~135ns ldweights)
                    for h in range(2):
                        cs = slice(h * 256, (h + 1) * 256)
                        sl = slice(joff + h * 256, joff + (h + 1) * 256)
                        nc.tensor.matmul(
                            psB_slice(p, 0)[:, cs], st, tl[:, sl],
                            start=(a == 0), stop=(a == KT - 1),
                        )
                else:
                    for b in range(blocks(p)):
                        sl = slice(joff + b * 512, joff + (b + 1) * 512)
                        nc.tensor.matmul(
                            psB_slice(p, b), st, tl[:, sl],
                            start=(a == 0), stop=(a == KT - 1),
                        )

            lt_ab = {}

            def emit_combine_ab(p, b):
                # psA stopped at end of pass A: fold its halves during pass B
                cab = work.tile([64, 512], f32, tag="c1")
                nc.scalar.activation(
                    out=cab, in_=psA[(p, b)][64:128, :],
                    func=mybir.ActivationFunctionType.Copy, scale=1.0,
                )
                ab = work.tile([64, 512], f32, tag="ltab", bufs=4)
                nc.vector.tensor_add(ab, cab, psA[(p, b)][0:64, :])
                lt_ab[(p, b)] = ab

            def emit_combine(p, b):
                # single DVE add on the post-passB critical path
                ltb = work.tile([64, 512], f32, tag="lt", bufs=4)
                nc.vector.tensor_add(ltb, lt_ab[(p, b)], psB_slice(p, b))
                lt[(p, b)] = ltb

            def emit_topk_tile(p, b, tt):
                # token tile index within the core
                t = (p_tok0[p] // 128) + b * 4 + tt
                ltb = lt[(p, b)]
                cs = slice(tt * 128, (tt + 1) * 128)
                ps_t = psT_pool.tile([128, E], f32, tag="ps_t")
                nc.tensor.transpose(ps_t, ltb[:, cs], id_t)
                m8 = work.tile([128, 8], f32, tag="m8")
                nc.vector.max(out=m8, in_=ps_t)
                nc.vector.max_index(stage_i[:, t * 8 : (t + 1) * 8], m8, ps_t)
                expw = work.tile([128, TOP_K], f32, tag="expw")
                ssum = work.tile([128, 1], f32, tag="ssum")
                nc.scalar.activation(
                    out=expw, in_=m8[:, 0:TOP_K],
                    func=mybir.ActivationFunctionType.Exp,
                    scale=LSCALE, accum_out=ssum[:, 0:1],
                )
                rsum = work.tile([128, 1], f32, tag="rsum")
                nc.vector.reciprocal(rsum, ssum)
                nc.vector.tensor_scalar_mul(
                    stage_w[:, t * TOP_K : (t + 1) * TOP_K], expw, rsum[:, 0:1]
                )

            def emit_out_dma(p):
                # output DMAs for panel p's token tiles, on the ACT queue
                c0 = p_tok0[p] // 128
                nt = PANELS[p] // 128
                nc.sync.dma_start(
                    out=out_w[:, c0 * TOP_K : (c0 + nt) * TOP_K],
                    in_=stage_w[:, c0 * TOP_K : (c0 + nt) * TOP_K],
                )
                nc.scalar.dma_start(
                    out=out_i[:, c0 * 8 : (c0 + nt) * 8],
                    in_=stage_i[:, c0 * 8 : (c0 + nt) * 8].bitcast(i32),
                )

            def drain_pending(n):
                for _ in range(n):
                    if pending_tiles:
                        pending_tiles.pop(0)()

            def psB_slice(p, b):
                return psB[p][b * 64 : (b + 1) * 64, :]

            for p in range(len(PANELS)):
                for b in range(blocks(p)):
                    psA[(p, b)] = psA_pool.tile([128, 512], f32, tag="psA", name=f"psA_{p}_{b}")
                psB[p] = psB_pool.tile([128, 512], f32, tag="psB", name=f"psB_{p}")

                for a in range(KT):
                    emit_passA_step(p, a)
                    if p == 0 and 1 <= a <= 13:
                        # early phase is DMA-bound: keep the PE clock warm;
                        # extra spins at chunk boundaries bridge the longer
                        # completion waits so the clock never demotes
                        n_j = 3 if a in (1, 5, 10) else 1
                        for _ in range(n_j):
                            nc.tensor.matmul(
                                ps_warm, junk[:, 0:128], junk, start=True, stop=True
                            )
                for a in range(KT):
                    emit_passB_step(p, a)
                    if a % 3 == 2:
                        drain_pending(1)
                    if a == 3:
                        for b in range(blocks(p)):
                            emit_combine_ab(p, b)

                for b in range(blocks(p)):
                    emit_combine(p, b)
                for b in range(blocks(p)):
                    for tt in range(4):
                        pending_tiles.append(
                            (lambda p=p, b=b, tt=tt: emit_topk_tile(p, b, tt))
                        )
                if p > 0:
                    prev = p - 1
                    pending_tiles.append(lambda prev=prev: emit_out_dma(prev))

            drain_pending(len(pending_tiles))
            emit_out_dma(len(PANELS) - 1)

    nc.compile()
    return nc


def _get_nc():
    if "nc" not in _CACHE:
        _CACHE["nc"] = _build()
    return _CACHE["nc"]


def kernel(hidden_states: np.ndarray, weight: np.ndarray, **_run_kwargs):
    x = np.ascontiguousarray(hidden_states, dtype=np.float32).reshape(T_FULL, H)
    w = np.ascontiguousarray(weight, dtype=np.float32)

    # fused weight stationary: [wh_a | wl_a * 2^11] per h-tile
    wh = w.astype(np.float16)
    wl = ((w - wh.astype(np.float32)) * 2048.0).astype(np.float16)
    whs = (wh.astype(np.float32) * 2048.0).astype(np.float16)  # exact pow2 scale
    ws = np.zeros((128, KT * 128), dtype=np.float16)
    wsb = np.zeros((128, KT * E), dtype=np.float16)
    for a in range(KT):
        ws[:, a * 128 : a * 128 + E] = whs[:, a * 128 : (a + 1) * 128].T
        ws[:, a * 128 + E : (a + 1) * 128] = wl[:, a * 128 : (a + 1) * 128].T
        wsb[:, a * E : (a + 1) * E] = wh[:, a * 128 : (a + 1) * 128].T
    ident = np.eye(E, dtype=np.float32)

    p_tok0 = [0, 1024, 1536]

    def pack_stream(xT, dt_):
        # [H, T_CORE] -> [128, KT*T_CORE] in stream order (panel, h-tile)
        out = np.empty((128, KT * T_CORE), dtype=dt_)
        col = 0
        for p, ptoks in enumerate(PANELS):
            t0 = p_tok0[p]
            blk = xT[:, t0 : t0 + ptoks].reshape(KT, 128, ptoks)
            out[:, col : col + KT * ptoks] = (
                blk.transpose(1, 0, 2).reshape(128, KT * ptoks)
            )
            col += KT * ptoks
        return out

    in_maps = []
    for c in range(N_CORES):
        shard = x[c * T_CORE : (c + 1) * T_CORE, :]
        xT = np.ascontiguousarray(shard.T)          # [H, T_CORE] f32
        xh16 = xT.astype(np.float16)
        xl8 = ((xT - xh16.astype(np.float32)) * 2048.0).astype(e4m3)
        in_maps.append(
            {
                "xh": pack_stream(xh16, np.float16),
                "xl": pack_stream(xl8, e4m3),
                "ws": ws,
                "wsb": wsb,
                "ident": ident,
            }
        )

    nc = _get_nc()
    res = run_bass_kernel_spmd(
        nc, in_maps, core_ids=list(range(N_CORES)), **_run_kwargs
    )

    idx_parts = []
    w_parts = []
    for c in range(N_CORES):
        r = res.results[c]
        si = r["out_i"].reshape(128, NTT, 8).transpose(1, 0, 2)[:, :, :TOP_K]
        sw = r["out_w"].reshape(128, NTT, TOP_K).transpose(1, 0, 2)
        idx_parts.append(si.reshape(T_CORE, TOP_K).astype(np.int32, copy=False))
        w_parts.append(sw.reshape(T_CORE, TOP_K))

    topk_idx = np.concatenate(idx_parts, axis=0)
    topk_weight = np.concatenate(w_parts, axis=0)
    if "trace" in _run_kwargs:
        return (topk_idx, topk_weight), res
    return topk_idx, topk_weight
